# revision 27
# baseline (speedup 1.0000x reference)
"""2-layer GCN (GCNConv+relu x2, linear head) on 8 Trainium2 NeuronCores.

Strategy (graph/data parallel, per sharding hint):
  - Nodes sharded across 8 cores by id; edges partitioned by destination.
  - Per core, destination nodes are bin-packed into B_FIX blocks of <=BLK
    dsts such that each (block, source-window) holds <= KCOL*128 edges.
    This gives an SPMD-uniform program; only tensor data varies per core.
  - Layer 1 runs in x-space: A(x W1) = (A x) W1, so the gather table is
    the dinv-scaled input x in bf16 ([slots, 128] rows = 256B), provided
    directly by the host to every core -- no device matmul and no
    AllGather before layer 1. Aggregation accumulates [128, BLK] blocks
    in PSUM via selection-matrix matmuls; W1 and W2 are applied per
    block afterwards, producing the layer-2 table rows (bf16, padded to
    128 cols so gather payloads stay 256B).
  - One AllGather (bf16) publishes the layer-2 table; layer 2 mirrors
    the baseline conv structure with bf16 message/selection matmuls.
  - dma_gather descriptor generation is the critical resource: the four
    source-window streams run on SWDGE queues 0-3, which execute on
    distinct Q7 core pairs concurrently (~4x the serial rate).
"""

import numpy as np

import concourse.bass as bass
import concourse.mybir as mybir
import concourse.tile as tile
from concourse import bacc
from concourse import bass_utils

import ml_dtypes

F32 = mybir.dt.float32
BF16 = mybir.dt.bfloat16
I16 = mybir.dt.int16
NP_BF16 = ml_dtypes.bfloat16


class Cfg:
    def __init__(self, n_nodes, in_feat, hidden, n_classes, n_cores, n_c,
                 blk, kcol, b_fix, nq, c_batch):
        self.N = n_nodes
        self.IN_FEAT = in_feat
        self.HIDDEN = hidden
        self.N_CLASSES = n_classes
        self.NC = n_cores
        self.N_C = n_c                    # nodes per core (id // N_C)
        assert n_c * n_cores >= n_nodes
        self.BLK = blk                    # max dsts per block
        self.KCOL = kcol                  # 128-edge columns per (block, stream)
        self.CAP = kcol * 128             # max edges per (block, stream)
        self.B_FIX = b_fix                # blocks per core (uniform)
        self.NQ = nq                      # source windows / gather streams
        self.SLOTS_C = b_fix * blk        # table slots per core
        assert self.SLOTS_C % 128 == 0
        self.NT = self.SLOTS_C // 128     # node tiles per core
        assert self.NT % 2 == 0
        self.TABLE_N = n_cores * self.SLOTS_C
        assert self.TABLE_N % nq == 0
        self.WIN = self.TABLE_N // nq     # table rows per source window
        assert self.WIN <= 32767          # int16 gather index range
        # quarter partitioning: stream q == source window q == quarter q of
        # every core's slot range, so the layer-2 table can be AllGathered in
        # four chunks that fire as soon as each quarter's rows are produced.
        assert n_c % nq == 0 and b_fix % nq == 0
        self.QN = n_c // nq               # nodes per (core, quarter)
        self.B_Q = b_fix // nq            # blocks per quarter
        self.WINC = self.B_Q * blk        # window rows contributed per core
        assert self.WINC % 128 == 0
        self.COLS_Q = b_fix * kcol        # gather columns per stream
        self.C_BATCH = c_batch            # columns per gather batch
        assert c_batch % kcol == 0 and self.COLS_Q % c_batch == 0
        self.N_BATCH = self.COLS_Q // c_batch
        self.BPB = c_batch // kcol        # blocks per batch
        assert self.BPB % 2 == 0          # block pairs never straddle batches


CFG_FULL = Cfg(n_nodes=100000, in_feat=128, hidden=64, n_classes=16,
               n_cores=8, n_c=12544, blk=64, kcol=2, b_fix=224, nq=4,
               c_batch=28)


# ---------------------------------------------------------------------------
# Host-side preprocessing (sharding): all integer graph restructuring.
# ---------------------------------------------------------------------------

def preprocess(cfg, x, edge_index, W1, b1, W2, b2, Wl, bl):
    N, NC, N_C = cfg.N, cfg.NC, cfg.N_C
    src = np.asarray(edge_index[0]).astype(np.int64)
    dst = np.asarray(edge_index[1]).astype(np.int64)
    x = np.asarray(x, dtype=np.float32)

    deg = np.bincount(dst, minlength=N).astype(np.float32) + 1.0
    dinv = (1.0 / np.sqrt(deg)).astype(np.float32)

    # stream of an edge = quarter of its source node within its core --
    # known before packing, and equal to the source's table window because
    # packing below keeps quarter-j nodes in quarter-j blocks.
    q_of = (src % N_C) // cfg.QN

    # per-(node, q) incoming edge counts
    degq = np.bincount(dst * cfg.NQ + q_of, minlength=N * cfg.NQ)\
             .reshape(N, cfg.NQ)

    # --- per-(core, quarter) first-fit-decreasing packing into blocks ---
    slot_of = np.full(NC * N_C, -1, dtype=np.int64)
    node_of_slot = np.full(cfg.TABLE_N, -1, dtype=np.int64)
    for c in range(NC):
        lo, hi = c * N_C, min((c + 1) * N_C, N)
        for jq in range(cfg.NQ):
            qlo, qhi = lo + jq * cfg.QN, min(lo + (jq + 1) * cfg.QN, hi)
            if qhi <= qlo:
                continue
            dq = degq[qlo:qhi]
            order = np.argsort(-dq.max(axis=1), kind="stable")
            accs = np.zeros((cfg.B_Q, cfg.NQ), dtype=np.int64)
            cnts = np.zeros(cfg.B_Q, dtype=np.int64)
            nopen = 1
            for j in order:
                v = dq[j]
                fits = (cnts[:nopen] < cfg.BLK) & \
                       np.all(accs[:nopen] + v <= cfg.CAP, axis=1)
                w = np.flatnonzero(fits)
                if w.size == 0:
                    assert nopen < cfg.B_Q, \
                        f"core {c} q {jq}: packing exceeds {cfg.B_Q} blocks"
                    b = nopen
                    nopen += 1
                else:
                    b = int(w[0])
                g = qlo + j
                bg = jq * cfg.B_Q + b
                s = c * cfg.SLOTS_C + bg * cfg.BLK + cnts[b]
                slot_of[g] = s
                node_of_slot[s] = g
                accs[b] += v
                cnts[b] += 1

    slot_of = slot_of[:N]

    # --- per-core edge streams ---
    e_core = dst // N_C
    s_slot = slot_of[src]
    d_slot_l = slot_of[dst] - e_core * cfg.SLOTS_C
    e_b = d_slot_l // cfg.BLK
    e_r = d_slot_l % cfg.BLK

    P_Q = cfg.B_FIX * cfg.CAP            # positions per stream
    idx_all = np.zeros((NC, cfg.NQ, P_Q), dtype=np.int16)
    dl_all = np.full((NC, cfg.NQ, P_Q), 255.0, dtype=np.float32)

    order2 = np.lexsort((e_b, q_of, e_core))
    es_c, eq_c, eb_c = e_core[order2], q_of[order2], e_b[order2]
    grp = (es_c * cfg.NQ + eq_c) * cfg.B_FIX + eb_c
    _, start_idx, cnt_grp = np.unique(grp, return_index=True,
                                      return_counts=True)
    rank = np.arange(grp.size) - np.repeat(start_idx, cnt_grp)
    assert rank.max(initial=0) < cfg.CAP
    pos = eb_c * cfg.CAP + rank
    # window-relative source index: window j rows are the concatenation of
    # every core's quarter-j slots (the order a chunked AllGather produces)
    ss = s_slot[order2]
    s_core = ss // cfg.SLOTS_C
    s_loc = ss % cfg.SLOTS_C
    s_qtr = s_loc // cfg.WINC
    assert np.all(s_qtr == eq_c), "stream != source window"
    idx_val = (s_core * cfg.WINC + s_loc - s_qtr * cfg.WINC).astype(np.int16)
    idx_all[es_c, eq_c, pos] = idx_val
    dl_all[es_c, eq_c, pos] = e_r[order2].astype(np.float32)

    # wrapped int16 layout: position i -> [i%16, i//16], replicated x8
    idx_w = idx_all.reshape(NC, cfg.NQ, -1, 16).transpose(0, 1, 3, 2)
    idx_dev = np.ascontiguousarray(np.tile(idx_w, (1, 1, 8, 1)))
    # dstloc layout: position -> [pos%128, pos//128]
    dl_dev = np.ascontiguousarray(
        dl_all.reshape(NC, cfg.NQ, cfg.COLS_Q, 128).transpose(0, 1, 3, 2))

    # --- per-slot node data: dinv-scaled x table in bf16 ---
    valid = node_of_slot >= 0
    xe = np.zeros((cfg.TABLE_N, cfg.IN_FEAT), dtype=np.float32)
    xe[valid] = x[node_of_slot[valid]] * dinv[node_of_slot[valid]][:, None]
    xe_bf = xe.astype(NP_BF16)
    # permute slot order -> window-major order [window j][core c][local]
    xtab = np.empty_like(xe_bf)
    for j in range(cfg.NQ):
        for c in range(NC):
            dst_o = j * cfg.WIN + c * cfg.WINC
            src_o = c * cfg.SLOTS_C + j * cfg.WINC
            xtab[dst_o:dst_o + cfg.WINC] = xe_bf[src_o:src_o + cfg.WINC]
    xtab = np.ascontiguousarray(xtab)
    dinv_s = np.zeros(cfg.TABLE_N, dtype=np.float32)
    dinv_s[valid] = dinv[node_of_slot[valid]]

    w1b = np.asarray(W1, np.float32).astype(NP_BF16)
    w2b = np.asarray(W2, np.float32).astype(NP_BF16)
    wlb = np.asarray(Wl, np.float32).astype(NP_BF16)
    b1 = np.asarray(b1, np.float32)
    b2 = np.asarray(b2, np.float32)
    bl = np.asarray(bl, np.float32)

    iota64 = np.tile(np.arange(cfg.BLK, dtype=np.float32)[None, :], (128, 1))
    ident2 = np.concatenate([np.eye(cfg.HIDDEN), np.eye(cfg.HIDDEN)],
                            axis=0).astype(NP_BF16)

    in_maps = []
    for c in range(NC):
        sl = slice(c * cfg.SLOTS_C, (c + 1) * cfg.SLOTS_C)
        dv = dinv_s[sl]
        # own rows in slot order, SBUF layout [128, NT*128]
        xself = np.ascontiguousarray(
            xe_bf[sl].reshape(cfg.NT, 128, cfg.IN_FEAT)
                     .transpose(1, 0, 2)
                     .reshape(128, cfg.NT * cfg.IN_FEAT))
        m = {
            "xtab": xtab,
            "xself": xself,
            "w1": w1b, "w2": w2b, "wl": wlb,
            "b1c": b1.reshape(-1, 1), "b2c": b2.reshape(-1, 1),
            "blrep": np.tile(bl[None, :], (128, 1)),
            "dinvn": np.ascontiguousarray(dv.reshape(cfg.NT, 128).T),
            "dinvfm": np.tile(dv[None, :], (128, 1)),
            "iota64": iota64,
            "ident2": ident2,
        }
        for q in range(cfg.NQ):
            m[f"idx{q}"] = idx_dev[c, q]
            m[f"dl{q}"] = dl_dev[c, q]
        in_maps.append(m)

    return in_maps, node_of_slot


def assemble_output(cfg, results, node_of_slot):
    out = np.zeros((cfg.N, cfg.N_CLASSES), dtype=np.float32)
    for c, r in enumerate(results):
        lg = r["logits"].reshape(128, cfg.NT, cfg.N_CLASSES)
        sl = node_of_slot[c * cfg.SLOTS_C:(c + 1) * cfg.SLOTS_C]\
            .reshape(cfg.NT, 128)
        for t in range(cfg.NT):
            v = sl[t] >= 0
            out[sl[t][v]] = lg[v, t, :]
    return out


# ---------------------------------------------------------------------------
# Device program
# ---------------------------------------------------------------------------

def build_program(cfg):
    nc = bacc.Bacc("TRN2", target_bir_lowering=False, debug=False,
                   num_devices=cfg.NC, num_swdge_queues=4)
    H, NT, F = cfg.HIDDEN, cfg.NT, cfg.IN_FEAT

    xtab_d = nc.dram_tensor("xtab", [cfg.TABLE_N, F], BF16,
                            kind="ExternalInput")
    xself_d = nc.dram_tensor("xself", [128, NT * F], BF16,
                             kind="ExternalInput")
    w1_d = nc.dram_tensor("w1", [F, H], BF16, kind="ExternalInput")
    w2_d = nc.dram_tensor("w2", [H, H], BF16, kind="ExternalInput")
    wl_d = nc.dram_tensor("wl", [H, cfg.N_CLASSES], BF16,
                          kind="ExternalInput")
    b1c_d = nc.dram_tensor("b1c", [H, 1], F32, kind="ExternalInput")
    b2c_d = nc.dram_tensor("b2c", [H, 1], F32, kind="ExternalInput")
    blrep_d = nc.dram_tensor("blrep", [128, cfg.N_CLASSES], F32,
                             kind="ExternalInput")
    dinvn_d = nc.dram_tensor("dinvn", [128, NT], F32, kind="ExternalInput")
    dinvfm_d = nc.dram_tensor("dinvfm", [128, cfg.SLOTS_C], F32,
                              kind="ExternalInput")
    iota_d = nc.dram_tensor("iota64", [128, cfg.BLK], F32,
                            kind="ExternalInput")
    ident_d = nc.dram_tensor("ident2", [128, H], BF16,
                             kind="ExternalInput")
    idx_d = [nc.dram_tensor(f"idx{q}", [128, cfg.COLS_Q * 8], I16,
                            kind="ExternalInput") for q in range(cfg.NQ)]
    dl_d = [nc.dram_tensor(f"dl{q}", [128, cfg.COLS_Q], F32,
                           kind="ExternalInput") for q in range(cfg.NQ)]
    logits_d = nc.dram_tensor("logits", [128, NT * cfg.N_CLASSES], F32,
                              kind="ExternalOutput")

    rg = [list(range(cfg.NC))]

    with tile.TileContext(nc) as tc:
        with tc.tile_pool(name="const", bufs=1) as cpool, \
             tc.tile_pool(name="dram", bufs=1, space="DRAM") as dpool, \
             tc.tile_pool(name="hp", bufs=4) as hpool:

            hs2w = [dpool.tile([cfg.WINC, F], BF16, tag=f"hs2w{j}",
                               name=f"hs2w{j}") for j in range(cfg.NQ)]
            tabw = [dpool.tile([cfg.WIN, F], BF16, tag=f"tabw{j}",
                               name=f"tabw{j}", addr_space="Shared")
                    for j in range(cfg.NQ)]

            def cload(dram, shape, dt, tag):
                t = cpool.tile(shape, dt, tag=tag)
                nc.sync.dma_start(out=t[:], in_=dram[:, :])
                return t

            # gather metadata first so queue desc-gen starts asap; stream 3
            # is first in the round order, so its indices load first
            idx_s, dl_s = [None] * cfg.NQ, [None] * cfg.NQ
            for q in (3, 0, 1, 2):
                idx_s[q] = cload(idx_d[q], [128, cfg.COLS_Q * 8], I16,
                                 f"idxs{q}")
            for q in (3, 0, 1, 2):
                dl_s[q] = cload(dl_d[q], [128, cfg.COLS_Q], F32, f"dls{q}")
            iota_s = cload(iota_d, [128, cfg.BLK], F32, "iota")
            xself_s = cload(xself_d, [128, NT * F], BF16, "xself")
            ident_s = cload(ident_d, [128, H], BF16, "ident")
            w1_s = cload(w1_d, [F, H], BF16, "w1")
            w2_s = cload(w2_d, [H, H], BF16, "w2")
            wl_s = cload(wl_d, [H, cfg.N_CLASSES], BF16, "wl")
            b1c_s = cload(b1c_d, [H, 1], F32, "b1c")
            b2c_s = cload(b2c_d, [H, 1], F32, "b2c")
            blrep_s = cload(blrep_d, [128, cfg.N_CLASSES], F32, "blrep")
            dinvn_s = cload(dinvn_d, [128, NT], F32, "dinvn")

            self2_s = cpool.tile([128, NT * H], BF16, tag="self2")

            with tc.tile_pool(name="sp", bufs=2) as spool, \
                 tc.tile_pool(name="dfp", bufs=2) as dfpool, \
                 tc.tile_pool(name="pagg", bufs=2, space="PSUM") as pagg, \
                 tc.tile_pool(name="p1p", bufs=2, space="PSUM") as p1p, \
                 tc.tile_pool(name="pp", bufs=2, space="PSUM") as pp:

                last_round_gathers = []

                def emit_ag(j):
                    cc = nc.gpsimd.collective_compute(
                        "AllGather", mybir.AluOpType.bypass,
                        replica_groups=rg,
                        ins=[hs2w[j].opt()], outs=[tabw[j].opt()])
                    # ordering-only edges: keep the Tile scheduler from
                    # hoisting the AG earlier in the Pool stream, where its
                    # input-row wait would stall gather desc-gen dispatch
                    for g in last_round_gathers:
                        bass._add_dep_helper(cc.ins, g.ins, sync=False,
                                             reason="ag placement")

                # L1 processes quarter 3 first so its AllGather fires while
                # the remaining quarters are still being gathered; stream
                # order within a round is [3, 0, 1, 2] to match AG order.
                QORD = [3, 0, 1, 2]

                def conv_layer(layer, batch_order, ag_at=None):
                    fw = F if layer == 1 else H      # message feature width
                    self_s = xself_s if layer == 1 else self2_s
                    bc_s = b1c_s if layer == 1 else b2c_s
                    pair = {}
                    for p, i in enumerate(batch_order):
                        if ag_at and p in ag_at:
                            emit_ag(ag_at[p])
                        msgs, Ss = {}, {}
                        round_gathers = []
                        for q in QORD:
                            if layer == 1:
                                win_ap = xtab_d[q * cfg.WIN:
                                                (q + 1) * cfg.WIN, :]
                            else:
                                win_ap = tabw[q][:, :]
                            msg_t = spool.tile([128, cfg.C_BATCH, F], BF16,
                                               tag=f"msg{q}", bufs=3)
                            g = nc.gpsimd.dma_gather(
                                out_ap=msg_t[:],
                                in_ap=win_ap,
                                idxs_ap=idx_s[q][:, i * cfg.C_BATCH * 8:
                                                 (i + 1) * cfg.C_BATCH * 8],
                                num_idxs=cfg.C_BATCH * 128,
                                num_idxs_reg=cfg.C_BATCH * 128,
                                elem_size=F, queue_num=q,
                                single_packet=False)
                            round_gathers.append(g)
                            S_t = spool.tile([128, cfg.C_BATCH, cfg.BLK],
                                             BF16, tag=f"S{q}")
                            iota_bc = iota_s[:]\
                                .rearrange("p (c f) -> p c f", c=1)\
                                .to_broadcast([128, cfg.C_BATCH, cfg.BLK])
                            dl_bc = dl_s[q][:, i * cfg.C_BATCH:
                                            (i + 1) * cfg.C_BATCH]\
                                .rearrange("p (c f) -> p c f", f=1)\
                                .to_broadcast([128, cfg.C_BATCH, cfg.BLK])
                            nc.vector.tensor_tensor(
                                out=S_t[:], in0=iota_bc, in1=dl_bc,
                                op=mybir.AluOpType.is_equal)
                            msgs[q] = msg_t[:].rearrange("p c f -> p (c f)")
                            Ss[q] = S_t[:].rearrange("p c f -> p (c f)")
                        last_round_gathers[:] = round_gathers

                        dfm_t = dfpool.tile([128, cfg.BPB * cfg.BLK], F32,
                                            tag="dfm")
                        nc.sync.dma_start(
                            out=dfm_t[:],
                            in_=dinvfm_d[:, i * cfg.BPB * cfg.BLK:
                                         (i + 1) * cfg.BPB * cfg.BLK])

                        for bb in range(cfg.BPB):
                            b = i * cfg.BPB + bb
                            half = (b % 2) * H
                            t = b // 2
                            pfm = pagg.tile([fw, cfg.BLK], F32, tag="fm")
                            nc.tensor.matmul(
                                out=pfm[:],
                                lhsT=self_s[half:half + H,
                                            t * fw:(t + 1) * fw],
                                rhs=ident_s[half:half + H, :],
                                start=True, stop=False)
                            for qi, q in enumerate(QORD):
                                for k in range(cfg.KCOL):
                                    lc = bb * cfg.KCOL + k
                                    last = (qi == cfg.NQ - 1 and
                                            k == cfg.KCOL - 1)
                                    nc.tensor.matmul(
                                        out=pfm[:],
                                        lhsT=msgs[q][:, lc * F:
                                                     lc * F + fw],
                                        rhs=Ss[q][:, lc * cfg.BLK:
                                                  (lc + 1) * cfg.BLK],
                                        start=False, stop=last)
                            if layer == 1:
                                # aggx = dinv_d * (A x)  -> bf16
                                aggx_t = hpool.tile([F, cfg.BLK], BF16,
                                                    tag="aggx")
                                nc.vector.tensor_tensor(
                                    out=aggx_t[:], in0=pfm[:],
                                    in1=dfm_t[:, bb * cfg.BLK:
                                              (bb + 1) * cfg.BLK],
                                    op=mybir.AluOpType.mult)
                                p1 = p1p.tile([H, cfg.BLK], F32, tag="p1")
                                nc.tensor.matmul(
                                    out=p1[:], lhsT=w1_s[:], rhs=aggx_t[:],
                                    start=True, stop=True)
                                hr_t = hpool.tile([H, cfg.BLK], BF16,
                                                  tag="hr")
                                nc.scalar.activation(
                                    out=hr_t[:], in_=p1[:],
                                    func=mybir.ActivationFunctionType.Relu,
                                    bias=bc_s[:])
                                if b % 2 == 0:
                                    pair["p2"] = pp.tile([128, H], F32,
                                                         name="p2",
                                                         tag="pair")
                                p2 = pair["p2"]
                                nc.tensor.matmul(
                                    out=p2[half:half + H, :], lhsT=hr_t[:],
                                    rhs=w2_s[:], start=True, stop=True,
                                    tile_position=(0, half))
                                if b % 2 == 1:
                                    dinv_bc = dinvn_s[:, t:t + 1]\
                                        .to_broadcast([128, H])
                                    row2 = hpool.tile([128, H], BF16,
                                                      tag="hs2row")
                                    nc.vector.tensor_tensor(
                                        out=row2[:], in0=p2[:], in1=dinv_bc,
                                        op=mybir.AluOpType.mult)
                                    jq = (t * 128) // cfg.WINC
                                    lr = t * 128 - jq * cfg.WINC
                                    nc.sync.dma_start(
                                        out=hs2w[jq][lr:lr + 128, 0:H],
                                        in_=row2[:])
                                    nc.vector.tensor_tensor(
                                        out=self2_s[:, t * H:(t + 1) * H],
                                        in0=p2[:], in1=dinv_bc,
                                        op=mybir.AluOpType.mult)
                            else:
                                h_t = hpool.tile([H, cfg.BLK], F32, tag="h")
                                nc.vector.tensor_tensor(
                                    out=h_t[:], in0=pfm[:],
                                    in1=dfm_t[0:H, bb * cfg.BLK:
                                              (bb + 1) * cfg.BLK],
                                    op=mybir.AluOpType.mult)
                                hr_t = hpool.tile([H, cfg.BLK], BF16,
                                                  tag="hr")
                                nc.scalar.activation(
                                    out=hr_t[:], in_=h_t[:],
                                    func=mybir.ActivationFunctionType.Relu,
                                    bias=bc_s[:])
                                if b % 2 == 0:
                                    pair["pl"] = pp.tile(
                                        [128, cfg.N_CLASSES], F32,
                                        name="pl", tag="pl")
                                pl = pair["pl"]
                                nc.tensor.matmul(
                                    out=pl[half:half + H, :], lhsT=hr_t[:],
                                    rhs=wl_s[:], start=True, stop=True,
                                    tile_position=(0, half))
                                if b % 2 == 1:
                                    nCL = cfg.N_CLASSES
                                    lg_t = hpool.tile([128, nCL], F32,
                                                      tag="lg")
                                    nc.vector.tensor_tensor(
                                        out=lg_t[:],
                                        in0=pl[:], in1=blrep_s[:],
                                        op=mybir.AluOpType.add)
                                    nc.sync.dma_start(
                                        out=logits_d[:, t * nCL:
                                                     (t + 1) * nCL],
                                        in_=lg_t[:])

                # quarter 3 batches first; each AG is placed ~4.5 rounds
                # after its quarter's gathers so the rows are already in
                # DRAM when the Pool sequencer reaches it (no stall).
                # AG_2 is emitted after layer 2's first round; queue 2
                # covers the AG_2 latency with its two prepared gathers.
                l1_order = [12, 13, 14, 15] + list(range(12))
                conv_layer(1, l1_order, ag_at={8: 3, 12: 0, 15: 1})
                conv_layer(2, list(range(cfg.N_BATCH)), ag_at={0: 2})


    nc.compile()
    return nc


_PROGRAM_CACHE = {}


def get_program(cfg):
    key = id(cfg)
    if key not in _PROGRAM_CACHE:
        _PROGRAM_CACHE[key] = build_program(cfg)
    return _PROGRAM_CACHE[key]


def run(cfg, inputs, trace=False):
    in_maps, node_of_slot = preprocess(cfg, **inputs)
    nc = get_program(cfg)
    res = bass_utils.run_bass_kernel_spmd(
        nc, in_maps, core_ids=list(range(cfg.NC)), trace=trace)
    out = assemble_output(cfg, res.results, node_of_slot)
    return out, res


def kernel(**inputs) -> np.ndarray:
    out, _ = run(CFG_FULL, inputs)
    return out


# revision 29
# speedup vs baseline: 1.0028x; 1.0028x over previous
"""2-layer GCN (GCNConv+relu x2, linear head) on 8 Trainium2 NeuronCores.

Strategy (graph/data parallel, per sharding hint):
  - Nodes sharded across 8 cores by id; edges partitioned by destination.
  - Per core, destination nodes are bin-packed into B_FIX blocks of <=BLK
    dsts such that each (block, source-window) holds <= KCOL*128 edges.
    This gives an SPMD-uniform program; only tensor data varies per core.
  - Layer 1 runs in x-space: A(x W1) = (A x) W1, so the gather table is
    the dinv-scaled input x in bf16 ([slots, 128] rows = 256B), provided
    directly by the host to every core -- no device matmul and no
    AllGather before layer 1. Aggregation accumulates [128, BLK] blocks
    in PSUM via selection-matrix matmuls; W1 and W2 are applied per
    block afterwards, producing the layer-2 table rows (bf16, padded to
    128 cols so gather payloads stay 256B).
  - One AllGather (bf16) publishes the layer-2 table; layer 2 mirrors
    the baseline conv structure with bf16 message/selection matmuls.
  - dma_gather descriptor generation is the critical resource: the four
    source-window streams run on SWDGE queues 0-3, which execute on
    distinct Q7 core pairs concurrently (~4x the serial rate).
"""

import numpy as np

import concourse.bass as bass
import concourse.mybir as mybir
import concourse.tile as tile
from concourse import bacc
from concourse import bass_utils

import ml_dtypes

F32 = mybir.dt.float32
BF16 = mybir.dt.bfloat16
I16 = mybir.dt.int16
NP_BF16 = ml_dtypes.bfloat16


class Cfg:
    def __init__(self, n_nodes, in_feat, hidden, n_classes, n_cores, n_c,
                 blk, kcol, b_fix, nq, c_batch):
        self.N = n_nodes
        self.IN_FEAT = in_feat
        self.HIDDEN = hidden
        self.N_CLASSES = n_classes
        self.NC = n_cores
        self.N_C = n_c                    # nodes per core (id // N_C)
        assert n_c * n_cores >= n_nodes
        self.BLK = blk                    # max dsts per block
        self.KCOL = kcol                  # 128-edge columns per (block, stream)
        self.CAP = kcol * 128             # max edges per (block, stream)
        self.B_FIX = b_fix                # blocks per core (uniform)
        self.NQ = nq                      # source windows / gather streams
        self.SLOTS_C = b_fix * blk        # table slots per core
        assert self.SLOTS_C % 128 == 0
        self.NT = self.SLOTS_C // 128     # node tiles per core
        assert self.NT % 2 == 0
        self.TABLE_N = n_cores * self.SLOTS_C
        assert self.TABLE_N % nq == 0
        self.WIN = self.TABLE_N // nq     # table rows per source window
        assert self.WIN <= 32767          # int16 gather index range
        # quarter partitioning: stream q == source window q == quarter q of
        # every core's slot range, so the layer-2 table can be AllGathered in
        # four chunks that fire as soon as each quarter's rows are produced.
        assert n_c % nq == 0 and b_fix % nq == 0
        self.QN = n_c // nq               # nodes per (core, quarter)
        self.B_Q = b_fix // nq            # blocks per quarter
        self.WINC = self.B_Q * blk        # window rows contributed per core
        assert self.WINC % 128 == 0
        self.COLS_Q = b_fix * kcol        # gather columns per stream
        self.C_BATCH = c_batch            # columns per gather batch
        assert c_batch % kcol == 0 and self.COLS_Q % c_batch == 0
        self.N_BATCH = self.COLS_Q // c_batch
        self.BPB = c_batch // kcol        # blocks per batch
        assert self.BPB % 2 == 0          # block pairs never straddle batches


CFG_FULL = Cfg(n_nodes=100000, in_feat=128, hidden=64, n_classes=16,
               n_cores=8, n_c=12544, blk=64, kcol=2, b_fix=224, nq=4,
               c_batch=28)


# ---------------------------------------------------------------------------
# Host-side preprocessing (sharding): all integer graph restructuring.
# ---------------------------------------------------------------------------

def preprocess(cfg, x, edge_index, W1, b1, W2, b2, Wl, bl):
    N, NC, N_C = cfg.N, cfg.NC, cfg.N_C
    src = np.asarray(edge_index[0]).astype(np.int64)
    dst = np.asarray(edge_index[1]).astype(np.int64)
    x = np.asarray(x, dtype=np.float32)

    deg = np.bincount(dst, minlength=N).astype(np.float32) + 1.0
    dinv = (1.0 / np.sqrt(deg)).astype(np.float32)

    # stream of an edge = quarter of its source node within its core --
    # known before packing, and equal to the source's table window because
    # packing below keeps quarter-j nodes in quarter-j blocks.
    q_of = (src % N_C) // cfg.QN

    # per-(node, q) incoming edge counts
    degq = np.bincount(dst * cfg.NQ + q_of, minlength=N * cfg.NQ)\
             .reshape(N, cfg.NQ)

    # --- per-(core, quarter) first-fit-decreasing packing into blocks ---
    slot_of = np.full(NC * N_C, -1, dtype=np.int64)
    node_of_slot = np.full(cfg.TABLE_N, -1, dtype=np.int64)
    for c in range(NC):
        lo, hi = c * N_C, min((c + 1) * N_C, N)
        for jq in range(cfg.NQ):
            qlo, qhi = lo + jq * cfg.QN, min(lo + (jq + 1) * cfg.QN, hi)
            if qhi <= qlo:
                continue
            dq = degq[qlo:qhi]
            order = np.argsort(-dq.max(axis=1), kind="stable")
            accs = np.zeros((cfg.B_Q, cfg.NQ), dtype=np.int64)
            cnts = np.zeros(cfg.B_Q, dtype=np.int64)
            nopen = 1
            for j in order:
                v = dq[j]
                fits = (cnts[:nopen] < cfg.BLK) & \
                       np.all(accs[:nopen] + v <= cfg.CAP, axis=1)
                w = np.flatnonzero(fits)
                if w.size == 0:
                    assert nopen < cfg.B_Q, \
                        f"core {c} q {jq}: packing exceeds {cfg.B_Q} blocks"
                    b = nopen
                    nopen += 1
                else:
                    b = int(w[0])
                g = qlo + j
                bg = jq * cfg.B_Q + b
                s = c * cfg.SLOTS_C + bg * cfg.BLK + cnts[b]
                slot_of[g] = s
                node_of_slot[s] = g
                accs[b] += v
                cnts[b] += 1

    slot_of = slot_of[:N]

    # --- per-core edge streams ---
    e_core = dst // N_C
    s_slot = slot_of[src]
    d_slot_l = slot_of[dst] - e_core * cfg.SLOTS_C
    e_b = d_slot_l // cfg.BLK
    e_r = d_slot_l % cfg.BLK

    P_Q = cfg.B_FIX * cfg.CAP            # positions per stream
    idx_all = np.zeros((NC, cfg.NQ, P_Q), dtype=np.int16)
    dl_all = np.full((NC, cfg.NQ, P_Q), 255.0, dtype=np.float32)

    order2 = np.lexsort((e_b, q_of, e_core))
    es_c, eq_c, eb_c = e_core[order2], q_of[order2], e_b[order2]
    grp = (es_c * cfg.NQ + eq_c) * cfg.B_FIX + eb_c
    _, start_idx, cnt_grp = np.unique(grp, return_index=True,
                                      return_counts=True)
    rank = np.arange(grp.size) - np.repeat(start_idx, cnt_grp)
    assert rank.max(initial=0) < cfg.CAP
    pos = eb_c * cfg.CAP + rank
    # window-relative source index: window j rows are the concatenation of
    # every core's quarter-j slots (the order a chunked AllGather produces)
    ss = s_slot[order2]
    s_core = ss // cfg.SLOTS_C
    s_loc = ss % cfg.SLOTS_C
    s_qtr = s_loc // cfg.WINC
    assert np.all(s_qtr == eq_c), "stream != source window"
    idx_val = (s_core * cfg.WINC + s_loc - s_qtr * cfg.WINC).astype(np.int16)
    idx_all[es_c, eq_c, pos] = idx_val
    dl_all[es_c, eq_c, pos] = e_r[order2].astype(np.float32)

    # wrapped int16 layout: position i -> [i%16, i//16], replicated x8
    idx_w = idx_all.reshape(NC, cfg.NQ, -1, 16).transpose(0, 1, 3, 2)
    idx_dev = np.ascontiguousarray(np.tile(idx_w, (1, 1, 8, 1)))
    # dstloc layout: position -> [pos%128, pos//128]
    dl_dev = np.ascontiguousarray(
        dl_all.reshape(NC, cfg.NQ, cfg.COLS_Q, 128).transpose(0, 1, 3, 2))

    # --- per-slot node data: dinv-scaled x table in bf16 ---
    valid = node_of_slot >= 0
    xe = np.zeros((cfg.TABLE_N, cfg.IN_FEAT), dtype=np.float32)
    xe[valid] = x[node_of_slot[valid]] * dinv[node_of_slot[valid]][:, None]
    xe_bf = xe.astype(NP_BF16)
    # permute slot order -> window-major order [window j][core c][local]
    xtab = np.empty_like(xe_bf)
    for j in range(cfg.NQ):
        for c in range(NC):
            dst_o = j * cfg.WIN + c * cfg.WINC
            src_o = c * cfg.SLOTS_C + j * cfg.WINC
            xtab[dst_o:dst_o + cfg.WINC] = xe_bf[src_o:src_o + cfg.WINC]
    xtab = np.ascontiguousarray(xtab)
    dinv_s = np.zeros(cfg.TABLE_N, dtype=np.float32)
    dinv_s[valid] = dinv[node_of_slot[valid]]

    w1b = np.asarray(W1, np.float32).astype(NP_BF16)
    w2b = np.asarray(W2, np.float32).astype(NP_BF16)
    wlb = np.asarray(Wl, np.float32).astype(NP_BF16)
    b1 = np.asarray(b1, np.float32)
    b2 = np.asarray(b2, np.float32)
    bl = np.asarray(bl, np.float32)

    iota64 = np.tile(np.arange(cfg.BLK, dtype=np.float32)[None, :], (128, 1))
    ident2 = np.concatenate([np.eye(cfg.HIDDEN), np.eye(cfg.HIDDEN)],
                            axis=0).astype(NP_BF16)

    in_maps = []
    for c in range(NC):
        sl = slice(c * cfg.SLOTS_C, (c + 1) * cfg.SLOTS_C)
        dv = dinv_s[sl]
        # own rows in slot order, SBUF layout [128, NT*128]
        xself = np.ascontiguousarray(
            xe_bf[sl].reshape(cfg.NT, 128, cfg.IN_FEAT)
                     .transpose(1, 0, 2)
                     .reshape(128, cfg.NT * cfg.IN_FEAT))
        m = {
            "xtab": xtab,
            "xself": xself,
            "w1": w1b, "w2": w2b, "wl": wlb,
            "b1c": b1.reshape(-1, 1), "b2c": b2.reshape(-1, 1),
            "blrep": np.tile(bl[None, :], (128, 1)),
            "dinvn": np.ascontiguousarray(dv.reshape(cfg.NT, 128).T),
            "dinvfm": np.tile(dv[None, :], (128, 1)),
            "iota64": iota64,
            "ident2": ident2,
        }
        for q in range(cfg.NQ):
            m[f"idx{q}"] = idx_dev[c, q]
            m[f"dl{q}"] = dl_dev[c, q]
        in_maps.append(m)

    return in_maps, node_of_slot


def assemble_output(cfg, results, node_of_slot):
    out = np.zeros((cfg.N, cfg.N_CLASSES), dtype=np.float32)
    for c, r in enumerate(results):
        lg = r["logits"].reshape(128, cfg.NT, cfg.N_CLASSES)
        sl = node_of_slot[c * cfg.SLOTS_C:(c + 1) * cfg.SLOTS_C]\
            .reshape(cfg.NT, 128)
        for t in range(cfg.NT):
            v = sl[t] >= 0
            out[sl[t][v]] = lg[v, t, :]
    return out


# ---------------------------------------------------------------------------
# Device program
# ---------------------------------------------------------------------------

def build_program(cfg):
    nc = bacc.Bacc("TRN2", target_bir_lowering=False, debug=False,
                   num_devices=cfg.NC, num_swdge_queues=4)
    H, NT, F = cfg.HIDDEN, cfg.NT, cfg.IN_FEAT

    xtab_d = nc.dram_tensor("xtab", [cfg.TABLE_N, F], BF16,
                            kind="ExternalInput")
    xself_d = nc.dram_tensor("xself", [128, NT * F], BF16,
                             kind="ExternalInput")
    w1_d = nc.dram_tensor("w1", [F, H], BF16, kind="ExternalInput")
    w2_d = nc.dram_tensor("w2", [H, H], BF16, kind="ExternalInput")
    wl_d = nc.dram_tensor("wl", [H, cfg.N_CLASSES], BF16,
                          kind="ExternalInput")
    b1c_d = nc.dram_tensor("b1c", [H, 1], F32, kind="ExternalInput")
    b2c_d = nc.dram_tensor("b2c", [H, 1], F32, kind="ExternalInput")
    blrep_d = nc.dram_tensor("blrep", [128, cfg.N_CLASSES], F32,
                             kind="ExternalInput")
    dinvn_d = nc.dram_tensor("dinvn", [128, NT], F32, kind="ExternalInput")
    dinvfm_d = nc.dram_tensor("dinvfm", [128, cfg.SLOTS_C], F32,
                              kind="ExternalInput")
    iota_d = nc.dram_tensor("iota64", [128, cfg.BLK], F32,
                            kind="ExternalInput")
    ident_d = nc.dram_tensor("ident2", [128, H], BF16,
                             kind="ExternalInput")
    idx_d = [nc.dram_tensor(f"idx{q}", [128, cfg.COLS_Q * 8], I16,
                            kind="ExternalInput") for q in range(cfg.NQ)]
    dl_d = [nc.dram_tensor(f"dl{q}", [128, cfg.COLS_Q], F32,
                           kind="ExternalInput") for q in range(cfg.NQ)]
    logits_d = nc.dram_tensor("logits", [128, NT * cfg.N_CLASSES], F32,
                              kind="ExternalOutput")

    rg = [list(range(cfg.NC))]

    with tile.TileContext(nc) as tc:
        with tc.tile_pool(name="const", bufs=1) as cpool, \
             tc.tile_pool(name="dram", bufs=1, space="DRAM") as dpool, \
             tc.tile_pool(name="hp", bufs=4) as hpool:

            hs2w = [dpool.tile([cfg.WINC, F], BF16, tag=f"hs2w{j}",
                               name=f"hs2w{j}") for j in range(cfg.NQ)]
            tabw = [dpool.tile([cfg.WIN, F], BF16, tag=f"tabw{j}",
                               name=f"tabw{j}", addr_space="Shared")
                    for j in range(cfg.NQ)]

            def cload(dram, shape, dt, tag):
                t = cpool.tile(shape, dt, tag=tag)
                nc.sync.dma_start(out=t[:], in_=dram[:, :])
                return t

            # gather metadata first so queue desc-gen starts asap; stream 3
            # is first in the round order, so its indices load first
            idx_s, dl_s = [None] * cfg.NQ, [None] * cfg.NQ
            for q in (3, 0, 1, 2):
                idx_s[q] = cload(idx_d[q], [128, cfg.COLS_Q * 8], I16,
                                 f"idxs{q}")
            for q in (3, 0, 1, 2):
                dl_s[q] = cload(dl_d[q], [128, cfg.COLS_Q], F32, f"dls{q}")
            iota_s = cload(iota_d, [128, cfg.BLK], F32, "iota")
            xself_s = cload(xself_d, [128, NT * F], BF16, "xself")
            ident_s = cload(ident_d, [128, H], BF16, "ident")
            w1_s = cload(w1_d, [F, H], BF16, "w1")
            w2_s = cload(w2_d, [H, H], BF16, "w2")
            wl_s = cload(wl_d, [H, cfg.N_CLASSES], BF16, "wl")
            b1c_s = cload(b1c_d, [H, 1], F32, "b1c")
            b2c_s = cload(b2c_d, [H, 1], F32, "b2c")
            blrep_s = cload(blrep_d, [128, cfg.N_CLASSES], F32, "blrep")
            dinvn_s = cload(dinvn_d, [128, NT], F32, "dinvn")

            self2_s = cpool.tile([128, NT * H], BF16, tag="self2")
            stageL_s = cpool.tile([128, NT * cfg.N_CLASSES], F32,
                                  tag="stgL")

            with tc.tile_pool(name="sp", bufs=2) as spool, \
                 tc.tile_pool(name="dfp", bufs=2) as dfpool, \
                 tc.tile_pool(name="pagg", bufs=2, space="PSUM") as pagg, \
                 tc.tile_pool(name="p1p", bufs=2, space="PSUM") as p1p, \
                 tc.tile_pool(name="pp", bufs=2, space="PSUM") as pp:

                last_round_gathers = []

                def emit_ag(j):
                    cc = nc.gpsimd.collective_compute(
                        "AllGather", mybir.AluOpType.bypass,
                        replica_groups=rg,
                        ins=[hs2w[j].opt()], outs=[tabw[j].opt()])
                    # ordering-only edges: keep the Tile scheduler from
                    # hoisting the AG earlier in the Pool stream, where its
                    # input-row wait would stall gather desc-gen dispatch
                    for g in last_round_gathers:
                        bass._add_dep_helper(cc.ins, g.ins, sync=False,
                                             reason="ag placement")

                # L1 processes quarter 3 first so its AllGather fires while
                # the remaining quarters are still being gathered; stream
                # order within a round is [3, 0, 1, 2] to match AG order.
                QORD = [3, 0, 1, 2]

                def conv_layer(layer, batch_order, ag_at=None):
                    fw = F if layer == 1 else H      # message feature width
                    self_s = xself_s if layer == 1 else self2_s
                    bc_s = b1c_s if layer == 1 else b2c_s
                    pair = {}
                    for p, i in enumerate(batch_order):
                        if ag_at and p in ag_at:
                            emit_ag(ag_at[p])
                        msgs, Ss = {}, {}
                        round_gathers = []
                        for q in QORD:
                            if layer == 1:
                                win_ap = xtab_d[q * cfg.WIN:
                                                (q + 1) * cfg.WIN, :]
                            else:
                                win_ap = tabw[q][:, :]
                            msg_t = spool.tile([128, cfg.C_BATCH, F], BF16,
                                               tag=f"msg{q}", bufs=3)
                            g = nc.gpsimd.dma_gather(
                                out_ap=msg_t[:],
                                in_ap=win_ap,
                                idxs_ap=idx_s[q][:, i * cfg.C_BATCH * 8:
                                                 (i + 1) * cfg.C_BATCH * 8],
                                num_idxs=cfg.C_BATCH * 128,
                                num_idxs_reg=cfg.C_BATCH * 128,
                                elem_size=F, queue_num=q,
                                single_packet=False)
                            round_gathers.append(g)
                            S_t = spool.tile([128, cfg.C_BATCH, cfg.BLK],
                                             BF16, tag=f"S{q}")
                            iota_bc = iota_s[:]\
                                .rearrange("p (c f) -> p c f", c=1)\
                                .to_broadcast([128, cfg.C_BATCH, cfg.BLK])
                            dl_bc = dl_s[q][:, i * cfg.C_BATCH:
                                            (i + 1) * cfg.C_BATCH]\
                                .rearrange("p (c f) -> p c f", f=1)\
                                .to_broadcast([128, cfg.C_BATCH, cfg.BLK])
                            nc.vector.tensor_tensor(
                                out=S_t[:], in0=iota_bc, in1=dl_bc,
                                op=mybir.AluOpType.is_equal)
                            msgs[q] = msg_t[:].rearrange("p c f -> p (c f)")
                            Ss[q] = S_t[:].rearrange("p c f -> p (c f)")
                        last_round_gathers[:] = round_gathers

                        dfm_t = dfpool.tile([128, cfg.BPB * cfg.BLK], F32,
                                            tag="dfm")
                        nc.sync.dma_start(
                            out=dfm_t[:],
                            in_=dinvfm_d[:, i * cfg.BPB * cfg.BLK:
                                         (i + 1) * cfg.BPB * cfg.BLK])

                        for bb in range(cfg.BPB):
                            b = i * cfg.BPB + bb
                            half = (b % 2) * H
                            t = b // 2
                            pfm = pagg.tile([fw, cfg.BLK], F32, tag="fm")
                            nc.tensor.matmul(
                                out=pfm[:],
                                lhsT=self_s[half:half + H,
                                            t * fw:(t + 1) * fw],
                                rhs=ident_s[half:half + H, :],
                                start=True, stop=False)
                            for qi, q in enumerate(QORD):
                                for k in range(cfg.KCOL):
                                    lc = bb * cfg.KCOL + k
                                    last = (qi == cfg.NQ - 1 and
                                            k == cfg.KCOL - 1)
                                    nc.tensor.matmul(
                                        out=pfm[:],
                                        lhsT=msgs[q][:, lc * F:
                                                     lc * F + fw],
                                        rhs=Ss[q][:, lc * cfg.BLK:
                                                  (lc + 1) * cfg.BLK],
                                        start=False, stop=last)
                            if layer == 1:
                                # aggx = dinv_d * (A x)  -> bf16
                                aggx_t = hpool.tile([F, cfg.BLK], BF16,
                                                    tag="aggx")
                                nc.vector.tensor_tensor(
                                    out=aggx_t[:], in0=pfm[:],
                                    in1=dfm_t[:, bb * cfg.BLK:
                                              (bb + 1) * cfg.BLK],
                                    op=mybir.AluOpType.mult)
                                p1 = p1p.tile([H, cfg.BLK], F32, tag="p1")
                                nc.tensor.matmul(
                                    out=p1[:], lhsT=w1_s[:], rhs=aggx_t[:],
                                    start=True, stop=True)
                                hr_t = hpool.tile([H, cfg.BLK], BF16,
                                                  tag="hr")
                                nc.scalar.activation(
                                    out=hr_t[:], in_=p1[:],
                                    func=mybir.ActivationFunctionType.Relu,
                                    bias=bc_s[:])
                                if b % 2 == 0:
                                    pair["p2"] = pp.tile([128, H], F32,
                                                         name="p2",
                                                         tag="pair")
                                p2 = pair["p2"]
                                nc.tensor.matmul(
                                    out=p2[half:half + H, :], lhsT=hr_t[:],
                                    rhs=w2_s[:], start=True, stop=True,
                                    tile_position=(0, half))
                                if b % 2 == 1:
                                    dinv_bc = dinvn_s[:, t:t + 1]\
                                        .to_broadcast([128, H])
                                    row2 = hpool.tile([128, H], BF16,
                                                      tag="hs2row")
                                    nc.vector.tensor_tensor(
                                        out=row2[:], in0=p2[:], in1=dinv_bc,
                                        op=mybir.AluOpType.mult)
                                    jq = (t * 128) // cfg.WINC
                                    lr = t * 128 - jq * cfg.WINC
                                    nc.sync.dma_start(
                                        out=hs2w[jq][lr:lr + 128, 0:H],
                                        in_=row2[:])
                                    nc.vector.tensor_tensor(
                                        out=self2_s[:, t * H:(t + 1) * H],
                                        in0=p2[:], in1=dinv_bc,
                                        op=mybir.AluOpType.mult)
                            else:
                                h_t = hpool.tile([H, cfg.BLK], F32, tag="h")
                                nc.vector.tensor_tensor(
                                    out=h_t[:], in0=pfm[:],
                                    in1=dfm_t[0:H, bb * cfg.BLK:
                                              (bb + 1) * cfg.BLK],
                                    op=mybir.AluOpType.mult)
                                hr_t = hpool.tile([H, cfg.BLK], BF16,
                                                  tag="hr")
                                nc.scalar.activation(
                                    out=hr_t[:], in_=h_t[:],
                                    func=mybir.ActivationFunctionType.Relu,
                                    bias=bc_s[:])
                                if b % 2 == 0:
                                    pair["pl"] = pp.tile(
                                        [128, cfg.N_CLASSES], F32,
                                        name="pl", tag="pl")
                                pl = pair["pl"]
                                nc.tensor.matmul(
                                    out=pl[half:half + H, :], lhsT=hr_t[:],
                                    rhs=wl_s[:], start=True, stop=True,
                                    tile_position=(0, half))
                                if b % 2 == 1:
                                    nCL = cfg.N_CLASSES
                                    nc.vector.tensor_tensor(
                                        out=stageL_s[:,
                                                     t * nCL:(t + 1) * nCL],
                                        in0=pl[:], in1=blrep_s[:],
                                        op=mybir.AluOpType.add)

                # quarter 3 batches first; each AG is placed ~4.5 rounds
                # after its quarter's gathers so the rows are already in
                # DRAM when the Pool sequencer reaches it (no stall).
                # AG_2 is emitted after layer 2's first round; queue 2
                # covers the AG_2 latency with its two prepared gathers.
                l1_order = [12, 13, 14, 15] + list(range(12))
                conv_layer(1, l1_order, ag_at={8: 3, 12: 0, 15: 1})
                conv_layer(2, list(range(cfg.N_BATCH)), ag_at={0: 2})

            nc.sync.dma_start(out=logits_d[:, :], in_=stageL_s[:])


    nc.compile()
    return nc


_PROGRAM_CACHE = {}


def get_program(cfg):
    key = id(cfg)
    if key not in _PROGRAM_CACHE:
        _PROGRAM_CACHE[key] = build_program(cfg)
    return _PROGRAM_CACHE[key]


def run(cfg, inputs, trace=False):
    in_maps, node_of_slot = preprocess(cfg, **inputs)
    nc = get_program(cfg)
    res = bass_utils.run_bass_kernel_spmd(
        nc, in_maps, core_ids=list(range(cfg.NC)), trace=trace)
    out = assemble_output(cfg, res.results, node_of_slot)
    return out, res


def kernel(**inputs) -> np.ndarray:
    out, _ = run(CFG_FULL, inputs)
    return out
